# revision 1
# baseline (speedup 1.0000x reference)
"""BiMamba block kernel for 8 Trainium2 NeuronCores (Bass/Tile, SPMD).

Sharding: core c -> (batch b = c//4, direction d = (c%4)//2, head-half hh = c%2).
Each core computes, for its (b, d, hh): rmsnorm -> in_proj slice -> causal conv
-> chunked SSD scan (16 local heads) -> gated partial products. The gated-RMSNorm
denominator and the output projection are returned as partial sums (P, ss) and
combined on the host (pure unshard/reduce, no cross-core collectives).

Chunked SSD (chunk C=128):
  L^T[s, (h,t)] = prod_{r=s+1..t} dA_r  built with ONE tensor_tensor_scan per
  chunk:  state = dA_rep * state + I_rep   (dA_rep col t=0 zeroed per head).
  M = G' (.) L^T with G'[s,t] = B_s.C_t;  Y_intra via PE (lhsT=X~, rhs=M);
  Y_inter accumulates into the same PSUM with rhs = C (.) beta; running state
  S_run decays by per-chunk Lambda; gamma (decay-to-end) = L^T[:, (h, C-1)].
All big matmuls run in float32r (~1.6e-4 rel err, 1 cycle/row).
"""

import numpy as np

import concourse.bass as bass
import concourse.mybir as mybir
import concourse.tile as tile
from concourse import bacc
from concourse.bass_utils import run_bass_kernel_spmd
from concourse.masks import make_identity

F32 = mybir.dt.float32
F32R = mybir.dt.float32r
AL = mybir.AluOpType
AF = mybir.ActivationFunctionType

DM = 1024          # d_model
H = 16             # local heads
P = 64             # head dim
N = 128            # state dim
CH = 1280          # conv channels (1024 x + 128 B + 128 C)
NIN = 2304         # in_proj cols: [x 1024 | B 128 | C 128 | z 1024], + dt 16
CK = 128           # chunk length
BLK = 512          # block length, 4 chunks
EPS = 1e-5

_cached = {}


def bcast_ap(dram_ap, parts=128):
    """Partition-broadcast read AP from a DRAM AP (stride-0 partition dim)."""
    return bass.AP(tensor=dram_ap.tensor, offset=dram_ap.offset,
                   ap=[[0, parts]] + list(dram_ap.ap))


def build_nc(L):
    NBLK = L // BLK
    NCH = L // CK
    nc = bacc.Bacc("TRN2", target_bir_lowering=False, debug=False)

    x_d = nc.dram_tensor("x", [L, DM], F32, kind="ExternalInput")
    win_d = nc.dram_tensor("w_in", [DM, NIN + 16], F32, kind="ExternalInput")
    wcomb_d = nc.dram_tensor("w_comb", [DM, 1024], F32, kind="ExternalInput")
    cw_d = nc.dram_tensor("conv_w", [CH, 4], F32, kind="ExternalInput")
    cb_d = nc.dram_tensor("conv_b", [CH, 1], F32, kind="ExternalInput")
    dtb_d = nc.dram_tensor("dt_bias", [H, 1], F32, kind="ExternalInput")
    an_d = nc.dram_tensor("a_neg", [H, 1], F32, kind="ExternalInput")
    dcol_d = nc.dram_tensor("d_col", [DM, 1], F32, kind="ExternalInput")

    pout_d = nc.dram_tensor("p_out", [1024, L], F32, kind="ExternalOutput")
    ss_d = nc.dram_tensor("ss_out", [1, L], F32, kind="ExternalOutput")

    with tile.TileContext(nc) as tc:
        with tc.tile_pool(name="dram", bufs=1, space="DRAM") as drp:
            xt_dram = drp.tile([L, DM], F32)       # X~ = dt-folded xs^T, L-major
            xs_dram = drp.tile([DM, L], F32)       # D-scaled xs, channel-major
            z_dram = drp.tile([DM, L], F32)        # z channel-major
            b_dram = drp.tile([N, L], F32)
            c_dram = drp.tile([N, L], F32)
            bt_dram = drp.tile([L, N], F32)        # B^T L-major
            da_dram = drp.tile([H, L], F32)        # dA = exp(dt*A)
            beta_dram = drp.tile([H, L], F32)      # decay-from-chunk-start
            lam_dram = drp.tile([H, NCH], F32)     # per-chunk total decay

            with tc.tile_pool(name="consts", bufs=1) as cons:
                ident = cons.tile([128, 128], F32)
                make_identity(nc, ident[:])
                ones_r = cons.tile([128, 1], F32R)
                nc.vector.memset(ones_r.bitcast(F32)[:], 1.0)
                cb_sb = cons.tile([128, CH // 128], F32)
                nc.sync.dma_start(cb_sb[:], cb_d.rearrange("(o p) k -> p (o k)", p=128))
                dtb_sb = cons.tile([H, 1], F32)
                nc.sync.dma_start(dtb_sb[:], dtb_d[:])
                an_sb = cons.tile([H, 1], F32)
                nc.sync.dma_start(an_sb[:], an_d[:])
                dcol_sb = cons.tile([128, DM // 128], F32)
                nc.sync.dma_start(dcol_sb[:], dcol_d.rearrange("(o p) k -> p (o k)", p=128))
                cw_sb = cons.tile([128, CH // 128, 4], F32)
                nc.sync.dma_start(cw_sb[:], cw_d.rearrange("(o p) k -> p o k", p=128))
                diag = cons.tile([128, CH // 128, 4, 128], F32R)
                for ct in range(CH // 128):
                    for k in range(4):
                        nc.vector.tensor_scalar_mul(
                            diag[:, ct, k, :], ident[:], cw_sb[:, ct, k:k + 1])
                one16 = cons.tile([H, 1], F32)
                nc.vector.memset(one16[:], 1.0)
                c_carry = cons.tile([H, 1], F32)
                nc.vector.memset(c_carry[:], 0.0)
                eps_col = cons.tile([128, 1], F32)
                nc.vector.memset(eps_col[:], EPS)
                ident_r = cons.tile([128, 128], F32R)
                nc.vector.tensor_copy(ident_r[:], ident[:])

                # ========= PHASE I: in_proj + conv + decay prep =========
                with tc.tile_pool(name="win", bufs=1) as winp:
                    w_sb = winp.tile([128, DM // 128, NIN], F32R)
                    wdt_sb = winp.tile([128, DM // 128, 16], F32R)
                    for d8 in range(DM // 128):
                        nc.sync.dma_start(
                            w_sb[:, d8, :],
                            win_d.bitcast(F32R)[d8 * 128:(d8 + 1) * 128, :NIN])
                        nc.sync.dma_start(
                            wdt_sb[:, d8, :],
                            win_d.bitcast(F32R)[d8 * 128:(d8 + 1) * 128, NIN:])

                    with tc.tile_pool(name="p1", bufs=2) as p1, \
                         tc.tile_pool(name="p1s1", bufs=1) as p1s1, \
                         tc.tile_pool(name="p1raw", bufs=1) as p1raw, \
                         tc.tile_pool(name="psin", bufs=2, space="PSUM") as psin, \
                         tc.tile_pool(name="pscv", bufs=2, space="PSUM") as pscv, \
                         tc.tile_pool(name="pstr", bufs=2, space="PSUM") as pstr, \
                         tc.tile_pool(name="pst2", bufs=2, space="PSUM") as pst2:
                        halo = p1.tile([128, CH // 128, 3], F32R, tag="halo")
                        for blk in range(NBLK):
                            l0 = blk * BLK
                            # rmsnorm + transpose -> xnT (fp32r)
                            xnt = p1s1.tile([128, DM // 128, BLK], F32R, tag="xnt")
                            for lt in range(BLK // 128):
                                xl = p1.tile([128, DM], F32, tag="xl")
                                nc.sync.dma_start(
                                    xl[:], x_d[l0 + lt * 128:l0 + (lt + 1) * 128, :])
                                sq = p1s1.tile([128, DM], F32, tag="sq")
                                ssc = p1.tile([128, 1], F32, tag="ssc")
                                nc.scalar.activation(sq[:], xl[:], AF.Square,
                                                     accum_out=ssc[:])
                                sc = p1.tile([128, 1], F32, tag="sc")
                                nc.scalar.activation(sc[:], ssc[:], AF.Sqrt,
                                                     bias=eps_col[:], scale=1.0 / DM)
                                nc.vector.reciprocal(sc[:], sc[:])
                                nc.vector.tensor_scalar_mul(xl[:], xl[:], sc[:])
                                for jg in range(2):
                                    pst = pstr.tile([128, 512], F32, tag="pst")
                                    for j4 in range(4):
                                        dj = jg * 4 + j4
                                        nc.tensor.transpose(
                                            pst[:, j4 * 128:(j4 + 1) * 128],
                                            xl[:, dj * 128:(dj + 1) * 128], ident[:])
                                    nc.vector.tensor_copy(
                                        xnt[:, jg * 4:(jg + 1) * 4,
                                            lt * 128:(lt + 1) * 128],
                                        pst.rearrange("p (a b) -> p a b", a=4)[:])
                            # in_proj
                            raw = p1raw.tile([128, CH // 128, BLK + 3], F32R, tag="raw")
                            if blk == 0:
                                nc.vector.memset(raw.bitcast(F32)[:, :, 0:3], 0.0)
                            else:
                                nc.vector.tensor_copy(raw[:, :, 0:3], halo[:])
                            for ct in range(18):
                                ps = psin.tile([128, BLK], F32, tag="ps_in")
                                for d8 in range(DM // 128):
                                    nc.tensor.matmul(
                                        ps[:], w_sb[:, d8, ct * 128:(ct + 1) * 128],
                                        xnt[:, d8, :],
                                        start=(d8 == 0), stop=(d8 == DM // 128 - 1))
                                if ct < 10:
                                    nc.scalar.activation(raw[:, ct, 3:], ps[:], AF.Copy)
                                else:
                                    sgz = p1.tile([128, BLK], F32, tag="sgz")
                                    nc.scalar.activation(sgz[:], ps[:], AF.Sigmoid)
                                    zt = p1.tile([128, BLK], F32, tag="zt")
                                    nc.vector.tensor_tensor(zt[:], ps[:], sgz[:], AL.mult)
                                    nc.sync.dma_start(
                                        z_dram[(ct - 10) * 128:(ct - 9) * 128,
                                               l0:l0 + BLK], zt[:])
                            psd = psin.tile([16, BLK], F32, tag="ps_in")
                            for d8 in range(DM // 128):
                                nc.tensor.matmul(
                                    psd[:], wdt_sb[:, d8, :], xnt[:, d8, :],
                                    start=(d8 == 0), stop=(d8 == DM // 128 - 1))
                            # dt chain
                            dt_sb = p1.tile([16, BLK], F32, tag="dt_sb")
                            nc.scalar.activation(dt_sb[:], psd[:], AF.Exp,
                                                 bias=dtb_sb[:])
                            nc.scalar.activation(dt_sb[:], dt_sb[:], AF.Ln,
                                                 bias=one16[:])
                            delta = p1.tile([16, BLK], F32, tag="delta")
                            nc.vector.tensor_scalar_mul(delta[:], dt_sb[:], an_sb[:])
                            da_sb = p1.tile([16, BLK], F32, tag="da_sb")
                            nc.scalar.activation(da_sb[:], delta[:], AF.Exp)
                            nc.sync.dma_start(da_dram[:, l0:l0 + BLK], da_sb[:])
                            zcol = p1.tile([16, BLK], F32, tag="zcol")
                            nc.vector.memset(zcol[:], 0.0)
                            c_sb = p1.tile([16, BLK], F32, tag="c_sb")
                            nc.vector.tensor_tensor_scan(
                                c_sb[:], delta[:], zcol[:], c_carry[:], AL.add, AL.add)
                            for kk in range(BLK // CK):
                                prevc = (c_carry[:] if kk == 0
                                         else c_sb[:, kk * CK - 1:kk * CK])
                                bl = p1.tile([16, CK], F32, tag="bl")
                                nc.vector.tensor_scalar_sub(
                                    bl[:], c_sb[:, kk * CK:(kk + 1) * CK], prevc)
                                nc.scalar.activation(bl[:], bl[:], AF.Exp)
                                nc.sync.dma_start(
                                    beta_dram[:, l0 + kk * CK:l0 + (kk + 1) * CK], bl[:])
                                lm = p1.tile([16, 1], F32, tag="lm")
                                nc.vector.tensor_scalar_sub(
                                    lm[:], c_sb[:, (kk + 1) * CK - 1:(kk + 1) * CK], prevc)
                                nc.scalar.activation(lm[:], lm[:], AF.Exp)
                                kidx = blk * (BLK // CK) + kk
                                nc.sync.dma_start(lam_dram[:, kidx:kidx + 1], lm[:])
                            nc.vector.tensor_copy(c_carry[:], c_sb[:, BLK - 1:BLK])
                            dtt = p1.tile([128, BLK // 128, 16], F32, tag="dtt")
                            for lt in range(BLK // 128):
                                pdt = pst2.tile([128, 16], F32, tag="ps_t")
                                nc.tensor.transpose(
                                    pdt[:], dt_sb[:, lt * 128:(lt + 1) * 128],
                                    ident[:16, :16])
                                nc.vector.tensor_copy(dtt[:, lt, :], pdt[:])
                            # conv + silu + layout staging
                            for ct in range(CH // 128):
                                psc = pscv.tile([128, BLK], F32, tag="ps_cv")
                                for k in range(4):
                                    nc.tensor.matmul(
                                        psc[:], diag[:, ct, 3 - k, :],
                                        raw[:, ct, 3 - k:3 - k + BLK],
                                        start=(k == 0), stop=(k == 3))
                                xbc = p1.tile([128, BLK], F32R, tag="xbc")
                                sg = p1.tile([128, BLK], F32, tag="sg")
                                nc.scalar.activation(sg[:], psc[:], AF.Sigmoid,
                                                     bias=cb_sb[:, ct:ct + 1])
                                nc.vector.scalar_tensor_tensor(
                                    xbc[:], psc[:], cb_sb[:, ct:ct + 1], sg[:],
                                    AL.add, AL.mult)
                                if ct < 8:
                                    xsd = p1.tile([128, BLK], F32, tag="xsd")
                                    nc.scalar.activation(xsd[:], xbc.bitcast(F32)[:],
                                                         AF.Copy,
                                                         scale=dcol_sb[:, ct:ct + 1])
                                    nc.sync.dma_start(
                                        xs_dram[ct * 128:(ct + 1) * 128, l0:l0 + BLK],
                                        xsd[:])
                                    for lt in range(BLK // 128):
                                        pxt = pst2.tile([128, 128], F32R, tag="ps_t")
                                        nc.tensor.transpose(
                                            pxt[:], xbc[:, lt * 128:(lt + 1) * 128],
                                            ident_r[:])
                                        xtt = p1.tile([128, 2, 64], F32, tag="xtt")
                                        h2 = 2 * ct
                                        nc.vector.tensor_tensor(
                                            xtt[:],
                                            pxt.bitcast(F32)
                                            .rearrange("p (a b) -> p a b", a=2)[:],
                                            dtt[:, lt, h2:h2 + 2, None]
                                            .to_broadcast((128, 2, 64)),
                                            AL.mult)
                                        nc.sync.dma_start(
                                            xt_dram[l0 + lt * 128:l0 + (lt + 1) * 128,
                                                    ct * 128:(ct + 1) * 128],
                                            xtt.rearrange("p a b -> p (a b)")[:])
                                elif ct == 8:
                                    nc.sync.dma_start(b_dram[:, l0:l0 + BLK],
                                                      xbc.bitcast(F32)[:])
                                    for lt in range(BLK // 128):
                                        pbt = pst2.tile([128, 128], F32R, tag="ps_t")
                                        nc.tensor.transpose(
                                            pbt[:], xbc[:, lt * 128:(lt + 1) * 128],
                                            ident_r[:])
                                        btt = p1.tile([128, 128], F32, tag="btt")
                                        nc.vector.tensor_copy(btt[:], pbt.bitcast(F32)[:])
                                        nc.sync.dma_start(
                                            bt_dram[l0 + lt * 128:l0 + (lt + 1) * 128, :],
                                            btt[:])
                                else:
                                    nc.sync.dma_start(c_dram[:, l0:l0 + BLK],
                                                      xbc.bitcast(F32)[:])
                            if blk < NBLK - 1:
                                nc.vector.tensor_copy(halo[:], raw[:, :, BLK:BLK + 3])

                # ========= PHASE II: chunked scan + gate + out proj =========
                with tc.tile_pool(name="wcb", bufs=1) as wcbp:
                    wc_sb = wcbp.tile([128, DM // 128, 1024], F32R)
                    for ft in range(DM // 128):
                        nc.sync.dma_start(
                            wc_sb[:, ft, :],
                            wcomb_d.bitcast(F32R)[ft * 128:(ft + 1) * 128, :])
                    with tc.tile_pool(name="p2", bufs=2) as p2, \
                         tc.tile_pool(name="p2s", bufs=1) as p2s, \
                         tc.tile_pool(name="p2g", bufs=1) as p2g, \
                         tc.tile_pool(name="psg", bufs=1, space="PSUM") as psgp, \
                         tc.tile_pool(name="psy", bufs=1, space="PSUM") as psyp, \
                         tc.tile_pool(name="pss", bufs=1, space="PSUM") as pssp, \
                         tc.tile_pool(name="pso", bufs=1, space="PSUM") as psop:
                        i_rep = p2s.tile([128, H * CK], F32)
                        for h in range(H):
                            nc.vector.tensor_copy(i_rep[:, h * CK:(h + 1) * CK], ident[:])
                        lamr = p2s.tile([128, H, NCH], F32)
                        nc.sync.dma_start(lamr[:], bcast_ap(lam_dram[:]))
                        s_run = p2s.tile([128, H * P], F32R)
                        nc.vector.memset(s_run.bitcast(F32)[:], 0.0)
                        for blk in range(NBLK):
                            l0 = blk * BLK
                            yg = p2g.tile([128, 8, BLK], F32R, tag="yg")
                            yg2 = p2g.tile([128, 8, BLK], F32R, tag="yg2")
                            for kk in range(BLK // CK):
                                k = blk * (BLK // CK) + kk
                                t0 = l0 + kk * CK
                                xt = p2.tile([128, H * P], F32R, tag="xt")
                                nc.sync.dma_start(
                                    xt[:], xt_dram.bitcast(F32R)[t0:t0 + CK, :])
                                bch = p2.tile([128, CK], F32R, tag="bch")
                                nc.sync.dma_start(
                                    bch[:], b_dram.bitcast(F32R)[:, t0:t0 + CK])
                                cch = p2.tile([128, CK], F32R, tag="cch")
                                nc.sync.dma_start(
                                    cch[:], c_dram.bitcast(F32R)[:, t0:t0 + CK])
                                btc = p2.tile([128, N], F32R, tag="btc")
                                nc.sync.dma_start(
                                    btc[:], bt_dram.bitcast(F32R)[t0:t0 + CK, :])
                                dar = p2.tile([128, H * CK], F32, tag="dar")
                                nc.sync.dma_start(
                                    dar[:], bcast_ap(da_dram[:, t0:t0 + CK]))
                                nc.vector.memset(
                                    dar.rearrange("p (h t) -> p h t", h=H)[:, :, 0:1],
                                    0.0)
                                btr = p2.tile([128, H * CK], F32R, tag="btr")
                                nc.sync.dma_start(
                                    btr[:],
                                    bcast_ap(beta_dram.bitcast(F32R)[:, t0:t0 + CK]))
                                lt_m = p2.tile([128, H * CK], F32R, tag="lt_m")
                                nc.vector.tensor_tensor_scan(
                                    lt_m[:], dar[:], i_rep[:], 0.0, AL.mult, AL.add)
                                xk = p2.tile([128, H * P], F32R, tag="xk")
                                nc.vector.tensor_tensor(
                                    xk.rearrange("p (h q) -> p h q", h=H)[:],
                                    xt.bitcast(F32).rearrange("p (h q) -> p h q", h=H)[:],
                                    lt_m.bitcast(F32).rearrange("p (h t) -> p h t", h=H)
                                    [:, :, CK - 1:CK].to_broadcast((128, H, P)),
                                    AL.mult)
                                psg = psgp.tile([128, CK], F32, tag="psg")
                                nc.tensor.matmul(psg[:], bch[:], cch[:],
                                                 start=True, stop=True)
                                nc.vector.tensor_tensor(
                                    lt_m.rearrange("p (h t) -> p h t", h=H)[:],
                                    lt_m.bitcast(F32).rearrange("p (h t) -> p h t", h=H)[:],
                                    psg[:, None, :].to_broadcast((128, H, CK)),
                                    AL.mult)
                                nc.vector.tensor_tensor(
                                    btr.rearrange("p (h t) -> p h t", h=H)[:],
                                    btr.bitcast(F32).rearrange("p (h t) -> p h t", h=H)[:],
                                    cch.bitcast(F32)[:, None, :]
                                    .to_broadcast((128, H, CK)),
                                    AL.mult)
                                psy = psyp.tile([64, H * CK], F32, tag="psy")
                                for h in range(H):
                                    po = psy[:, h * CK:(h + 1) * CK]
                                    nc.tensor.matmul(
                                        po, xt[:, h * P:(h + 1) * P],
                                        lt_m[:, h * CK:(h + 1) * CK],
                                        start=True, stop=(k == 0))
                                    if k > 0:
                                        nc.tensor.matmul(
                                            po, s_run[:, h * P:(h + 1) * P],
                                            btr[:, h * CK:(h + 1) * CK],
                                            start=False, stop=True)
                                pss = pssp.tile([128, H * P], F32, tag="pss")
                                for h in range(H):
                                    nc.tensor.matmul(
                                        pss[:, h * P:(h + 1) * P], btc[:],
                                        xk[:, h * P:(h + 1) * P],
                                        start=True, stop=True)
                                nc.vector.tensor_tensor(
                                    s_run.rearrange("p (h q) -> p h q", h=H)[:],
                                    s_run.bitcast(F32).rearrange("p (h q) -> p h q", h=H)[:],
                                    lamr[:, :, k:k + 1].to_broadcast((128, H, P)),
                                    AL.mult)
                                nc.vector.tensor_tensor(
                                    s_run[:], s_run.bitcast(F32)[:], pss[:], AL.add)
                                # y + gate
                                xsc = p2.tile([128, 8, CK], F32, tag="xsc")
                                nc.sync.dma_start(
                                    xsc[:], xs_dram.rearrange("(a p) l -> p a l", p=128)
                                    [:, :, t0:t0 + CK])
                                ych = p2.tile([128, 8, CK], F32, tag="ych")
                                psy3 = psy.rearrange("p (a c b) -> p a c b", a=8, c=2)
                                nc.vector.tensor_tensor(
                                    ych[0:64], psy3[:, :, 0, :], xsc[0:64], AL.add)
                                sbo = p2.tile([64, 8, CK], F32, tag="sbo")
                                nc.scalar.activation(sbo[:], psy3[:, :, 1, :], AF.Copy)
                                nc.sync.dma_start(ych[64:128], sbo[:])
                                nc.vector.tensor_tensor(
                                    ych[64:128], ych[64:128], xsc[64:128], AL.add)
                                zch = p2.tile([128, 8, CK], F32, tag="zch")
                                nc.sync.dma_start(
                                    zch[:], z_dram.rearrange("(a p) l -> p a l", p=128)
                                    [:, :, t0:t0 + CK])
                                nc.vector.tensor_tensor(
                                    yg[:, :, kk * CK:(kk + 1) * CK], ych[:], zch[:],
                                    AL.mult)
                                nc.scalar.activation(
                                    yg2[:, :, kk * CK:(kk + 1) * CK],
                                    yg.bitcast(F32)[:, :, kk * CK:(kk + 1) * CK],
                                    AF.Square)
                            # ss reduce + Wcomb partial product
                            psl = psop.tile([1, BLK], F32, tag="po")
                            for ft in range(8):
                                nc.tensor.matmul(psl[:], ones_r[:], yg2[:, ft, :],
                                                 start=(ft == 0), stop=(ft == 7))
                            ssb = p2.tile([1, BLK], F32, tag="ssb")
                            nc.vector.tensor_copy(ssb[:], psl[:])
                            nc.sync.dma_start(ss_d[:, l0:l0 + BLK], ssb[:])
                            for ot in range(8):
                                pso = psop.tile([128, BLK], F32, tag="po")
                                for ft in range(8):
                                    nc.tensor.matmul(
                                        pso[:], wc_sb[:, ft, ot * 128:(ot + 1) * 128],
                                        yg[:, ft, :],
                                        start=(ft == 0), stop=(ft == 7))
                                ob = p2.tile([128, BLK], F32, tag="ob")
                                nc.vector.tensor_copy(ob[:], pso[:])
                                nc.sync.dma_start(
                                    pout_d[ot * 128:(ot + 1) * 128, l0:l0 + BLK], ob[:])
    nc.compile()
    return nc


def _prep_core_inputs(inputs, b, d, hh, L):
    pre = "fwd_" if d == 0 else "bwd_"
    W_in = inputs[pre + "W_in"]
    W_out = inputs[pre + "W_out"]
    conv_w = inputs[pre + "conv_w"]
    conv_b = inputs[pre + "conv_b"]
    dt_bias = inputs[pre + "dt_bias"]
    A_log = inputs[pre + "A_log"]
    Dp = inputs[pre + "D"]
    norm_w = inputs[pre + "norm_w"]
    proj_W = inputs["proj_W"]

    xrows = W_in[2048 + hh * 1024:2048 + (hh + 1) * 1024]
    Brows = W_in[4096:4224]
    Crows = W_in[4224:4352]
    zrows = W_in[hh * 1024:(hh + 1) * 1024]
    dtrows = W_in[4352 + hh * 16:4352 + hh * 16 + 16]
    w_inT = np.ascontiguousarray(
        np.concatenate([xrows, Brows, Crows, zrows, dtrows], 0).T)

    wcomb = np.ascontiguousarray(
        (norm_w[hh * 1024:(hh + 1) * 1024, None].astype(np.float64)
         * W_out[:, hh * 1024:(hh + 1) * 1024].T.astype(np.float64)
         @ proj_W[:, d * 1024:(d + 1) * 1024].T.astype(np.float64)).astype(np.float32))

    cw = np.ascontiguousarray(
        np.concatenate([conv_w[hh * 1024:(hh + 1) * 1024], conv_w[2048:2304]], 0))
    cb = np.ascontiguousarray(
        np.concatenate([conv_b[hh * 1024:(hh + 1) * 1024], conv_b[2048:2304]], 0))

    x_in = inputs["x"][b]
    if d == 1:
        x_in = x_in[::-1]
    dcol = np.repeat(Dp[hh * 16:hh * 16 + 16], 64)

    return {
        "x": np.ascontiguousarray(x_in[:L], np.float32),
        "w_in": w_inT.astype(np.float32),
        "w_comb": wcomb,
        "conv_w": cw.astype(np.float32),
        "conv_b": cb.reshape(-1, 1).astype(np.float32),
        "dt_bias": dt_bias[hh * 16:hh * 16 + 16].reshape(-1, 1).astype(np.float32),
        "a_neg": (-np.exp(A_log[hh * 16:hh * 16 + 16])).reshape(-1, 1).astype(np.float32),
        "d_col": dcol.reshape(-1, 1).astype(np.float32),
    }


def kernel(**inputs):
    L = 2048
    inputs = {k: np.asarray(v) for k, v in inputs.items()}
    if "nc" not in _cached:
        _cached["nc"] = build_nc(L)
    nc = _cached["nc"]

    in_maps = [
        _prep_core_inputs(inputs, c // 4, (c % 4) // 2, c % 2, L) for c in range(8)
    ]
    res = run_bass_kernel_spmd(nc, in_maps, core_ids=list(range(8)))

    x = inputs["x"]
    out = x.astype(np.float64) + inputs["proj_b"].astype(np.float64)[None, None, :]
    for b in range(2):
        for d in range(2):
            c0 = b * 4 + d * 2
            ss = (res.results[c0]["ss_out"][0].astype(np.float64)
                  + res.results[c0 + 1]["ss_out"][0].astype(np.float64))
            s = 1.0 / np.sqrt(ss / 2048.0 + EPS)
            Psum = (res.results[c0]["p_out"].astype(np.float64)
                    + res.results[c0 + 1]["p_out"].astype(np.float64))
            contrib = (s[None, :] * Psum).T
            if d == 1:
                contrib = contrib[::-1]
            out[b] += contrib
    return out.astype(np.float32)

